# revision 3
# baseline (speedup 1.0000x reference)
"""Batched sparse forward projection Y[b,r] = sum_k vals[k]*X[b,cols[k]] for rows[k]==r.

Strategy (8 NeuronCores, row-sharded):
- Each core owns a 16384-row slice of the output; its nnz slice is found by
  searchsorted on the (sorted) rows array.
- Within a core, nnz are bucketed by col>>13 into 8 buckets, one per GPSIMD Q7
  core (partitions 16c..16c+15). Stable bucketing keeps rows sorted per bucket.
- The gather X.T[col] runs on GPSIMD via ap_gather: the X table lives in SBUF as
  [128 channels, 8192, 1] f32 where channel 16c+j (j<8) holds X[j, 8192c+e].
  Each Q7 core gathers with its own (wrapped) int16 index list, fetching all 8
  batch values per nnz in one index.
- Slots are padded per (row, bucket) to multiples of K=4 (idx=-1 gathers elem 0,
  val=0). A strided DVE reduce produces per-4-slot subtotals; a 4-pass masked
  Hillis-Steele segmented scan (masks from an is_equal on a row-id stream)
  turns them into inclusive per-row prefixes; a second small ap_gather extracts
  each row's last subtotal (= the row total per bucket and batch).
- A single [128,8] selection matmul sums the 8 buckets into PSUM [8, rows],
  which is copied to an [8, 16384] staging tile and DMA'd out.
"""

import numpy as np

import concourse.bass as bass
import concourse.mybir as mybir
import concourse.tile as tile
from concourse import bacc
from concourse.bass_utils import run_bass_kernel_spmd

B = 8
N_PIX = 65536
N_ROWS = 131072
N_CORES = 8
NBUK = 8  # col buckets = Q7 cores
BUK = N_PIX // NBUK  # 8192 pixels per bucket
K = 4  # slots per subtotal group
P = 128

_compiled = {}


def _ceil_to(x, m):
    return -(-x // m) * m


def _prep_core(rows_l, cols_n, vals_n, rows_per_core, rpc):
    """Build per-core packed arrays. rows_l: local row ids (sorted), cols_n, vals_n.

    rpc: rows per chunk (mult of 16). Returns dict of arrays + per-chunk needs.
    Layout is finalized later (shared CLs across cores)."""
    nnz = rows_l.shape[0]
    buk = (cols_n >> 13).astype(np.int64)
    e = (cols_n & (BUK - 1)).astype(np.int16)
    key = buk * rows_per_core + rows_l.astype(np.int64)
    perm = np.argsort(key, kind="stable")
    skey = key[perm]
    cnt = np.bincount(key, minlength=NBUK * rows_per_core).reshape(NBUK, rows_per_core)
    grp = np.maximum(1, -(-cnt // K))  # groups per (bucket,row), >=1
    assert grp.max() <= 16, f"row too heavy for 4-pass scan: {grp.max()} groups"
    pad_slots = grp * K  # padded slots per (bucket,row)

    n_chunks = -(-rows_per_core // rpc)
    # per-chunk slot needs per bucket
    need = np.zeros((NBUK, n_chunks), np.int64)
    for k in range(n_chunks):
        r0, r1 = k * rpc, min((k + 1) * rpc, rows_per_core)
        need[:, k] = pad_slots[:, r0:r1].sum(axis=1)
    return {
        "perm": perm,
        "skey": skey,
        "cnt": cnt,
        "grp": grp,
        "pad_slots": pad_slots,
        "e": e,
        "vals": vals_n,
        "need": need,
        "n_chunks": n_chunks,
    }


def _layout_core(prep, cls_, rows_per_core, rpc):
    """Given shared per-chunk CLs, build the packed device arrays for one core."""
    NBUKl = NBUK
    n_chunks = len(cls_)
    Ltot = int(sum(cls_))
    grp, pad_slots, cnt = prep["grp"], prep["pad_slots"], prep["cnt"]

    # global padded start offset of each (bucket,row) run
    off = np.zeros((NBUKl, rows_per_core), np.int64)
    base = 0
    for k in range(n_chunks):
        r0, r1 = k * rpc, min((k + 1) * rpc, rows_per_core)
        ps = pad_slots[:, r0:r1]
        cs = np.cumsum(ps, axis=1)
        off[:, r0:r1] = base + cs - ps
        base += cls_[k]
    assert base == Ltot

    idx16 = np.full((NBUKl, Ltot), -1, np.int16)
    valsd = np.zeros((NBUKl, Ltot), np.float32)

    # scatter real nnz into padded slots
    skey, perm = prep["skey"], prep["perm"]
    cnt_flat = cnt.reshape(-1)
    starts_flat = np.cumsum(cnt_flat) - cnt_flat  # start of each (c,r) in sorted stream
    o = np.arange(skey.shape[0], dtype=np.int64) - starts_flat[skey]
    c_sorted = skey // rows_per_core
    r_sorted = skey % rows_per_core
    pos = off[c_sorted, r_sorted] + o
    idx16[c_sorted, pos] = prep["e"][perm]
    valsd[c_sorted, pos] = prep["vals"][perm]

    # rowid stream (per group) and end positions (chunk-relative group index)
    NGtot = Ltot // K
    rowid = np.full((NBUKl, NGtot), -1.0, np.float32)
    endpos = np.zeros((NBUKl, rows_per_core), np.int16)
    gbase = 0
    for k in range(n_chunks):
        r0, r1 = k * rpc, min((k + 1) * rpc, rows_per_core)
        ng_k = cls_[k] // K
        for c in range(NBUKl):
            g = grp[c, r0:r1]
            rid = np.repeat(np.arange(r0, r1, dtype=np.float32), g)
            rowid[c, gbase : gbase + rid.shape[0]] = rid
            ge = np.cumsum(g)  # end (exclusive) group idx within chunk
            endpos[c, r0:r1] = (ge - 1).astype(np.int16)
        gbase += ng_k
    assert gbase == NGtot

    # wrapped views for per-Q7-core index lists
    idxw = np.ascontiguousarray(
        idx16.reshape(NBUKl, Ltot // 16, 16).transpose(0, 2, 1)
    ).reshape(NBUKl * 16, Ltot // 16)
    epw = np.ascontiguousarray(
        endpos.reshape(NBUKl, rows_per_core // 16, 16).transpose(0, 2, 1)
    ).reshape(NBUKl * 16, rows_per_core // 16)
    return {"idxw": idxw, "valsd": valsd, "rowid": rowid, "epw": epw}


def _build_nc(cls_, rpc, rows_per_core, bcast_dma):
    """Build + compile the Bass program for the shared chunk layout."""
    n_chunks = len(cls_)
    Ltot = int(sum(cls_))
    NGtot = Ltot // K
    nc = bacc.Bacc("TRN2", target_bir_lowering=False, debug=False)
    f32, i16 = mybir.dt.float32, mybir.dt.int16

    xt = nc.dram_tensor("xt", [P, BUK], f32, kind="ExternalInput")
    sel = nc.dram_tensor("sel", [P, B], f32, kind="ExternalInput")
    idxw = nc.dram_tensor("idxw", [P, Ltot // 16], i16, kind="ExternalInput")
    valsd = nc.dram_tensor("valsd", [NBUK, Ltot], f32, kind="ExternalInput")
    rowidd = nc.dram_tensor("rowidd", [NBUK, NGtot], f32, kind="ExternalInput")
    epw = nc.dram_tensor("epw", [P, rows_per_core // 16], i16, kind="ExternalInput")
    y = nc.dram_tensor("y", [B, rows_per_core], f32, kind="ExternalOutput")

    with tile.TileContext(nc) as tc:
        with (
            tc.tile_pool(name="tabp", bufs=1) as tabp,
            tc.tile_pool(name="selp", bufs=1) as selp,
            tc.tile_pool(name="idxp", bufs=2) as idxp,
            tc.tile_pool(name="gtp", bufs=1) as gtp,
            tc.tile_pool(name="valp", bufs=2) as valp,
            tc.tile_pool(name="rowp", bufs=1) as rowp,
            tc.tile_pool(name="scanp", bufs=2) as scanp,
            tc.tile_pool(name="maskp", bufs=1) as maskp,
            tc.tile_pool(name="epp", bufs=2) as epp,
            tc.tile_pool(name="etp", bufs=2) as etp,
            tc.tile_pool(name="ysbp", bufs=2) as ysbp,
            tc.tile_pool(name="psp", bufs=2, space="PSUM") as psp,
        ):
            tab_t = tabp.tile([P, BUK, 1], f32)
            nc.sync.dma_start(tab_t[:, :, 0], xt[:])
            sel_t = selp.tile([P, B], f32)
            nc.sync.dma_start(sel_t[:], sel[:])

            sbase = 0  # slot base
            for k in range(n_chunks):
                CL = int(cls_[k])
                NG = CL // K
                rb = k * rpc
                rpck = min(rpc, rows_per_core - rb)

                it = idxp.tile([P, CL // 16], i16, tag="idx")
                nc.sync.dma_start(
                    it[:], idxw[:, sbase // 16 : (sbase + CL) // 16]
                )
                gt = gtp.tile([P, CL, 1], f32, tag="gt")
                nc.gpsimd.ap_gather(
                    out_ap=gt[:],
                    in_ap=tab_t[:],
                    idxs_ap=it[:],
                    channels=P,
                    num_elems=BUK,
                    d=1,
                    num_idxs=CL,
                )
                rt = rowp.tile([P, NG], f32, tag="row")
                for c in range(NBUK):
                    src_r = bass.AP(
                        rowidd, c * NGtot + sbase // K, [[0, 16], [1, NG]]
                    )
                    nc.sync.dma_start(rt[16 * c : 16 * c + 16, :], src_r)
                # contrib = gathered * val (in place over gt), in sub-tiles
                VS = 4096
                for sb0 in range(0, CL, VS):
                    vs = min(VS, CL - sb0)
                    vt = valp.tile([P, VS], f32, tag="val")
                    for c in range(NBUK):
                        src_v = bass.AP(
                            valsd, c * Ltot + sbase + sb0, [[0, 16], [1, vs]]
                        )
                        nc.sync.dma_start(vt[16 * c : 16 * c + 16, :vs], src_v)
                    nc.vector.tensor_tensor(
                        out=gt[:, sb0 : sb0 + vs, 0],
                        in0=gt[:, sb0 : sb0 + vs, 0],
                        in1=vt[:, :vs],
                        op=mybir.AluOpType.mult,
                    )
                # subtotal per K consecutive slots
                sub = scanp.tile([P, NG, 1], f32, tag="scan")
                nc.vector.tensor_reduce(
                    out=sub[:, :, 0],
                    in_=gt[:, :, 0].rearrange("p (g k) -> p g k", k=K),
                    axis=mybir.AxisListType.X,
                    op=mybir.AluOpType.add,
                )
                # masked segmented scan over groups
                cur = sub
                s = 1
                while s < 16:
                    mk = maskp.tile([P, NG, 1], f32, tag="mask")
                    nc.vector.tensor_tensor(
                        out=mk[:, s:NG, 0],
                        in0=rt[:, s:NG],
                        in1=rt[:, 0 : NG - s],
                        op=mybir.AluOpType.is_equal,
                    )
                    nc.vector.tensor_tensor(
                        out=mk[:, s:NG, 0],
                        in0=cur[:, 0 : NG - s, 0],
                        in1=mk[:, s:NG, 0],
                        op=mybir.AluOpType.mult,
                    )
                    nxt = scanp.tile([P, NG, 1], f32, tag="scan")
                    nc.vector.tensor_copy(out=nxt[:, :s, 0], in_=cur[:, :s, 0])
                    nc.vector.tensor_tensor(
                        out=nxt[:, s:NG, 0],
                        in0=cur[:, s:NG, 0],
                        in1=mk[:, s:NG, 0],
                        op=mybir.AluOpType.add,
                    )
                    cur = nxt
                    s *= 2
                # extract per-row totals (last group of each row)
                ep = epp.tile([P, rpck // 16], i16, tag="ep")
                nc.sync.dma_start(ep[:], epw[:, rb // 16 : (rb + rpck) // 16])
                et = etp.tile([P, rpck, 1], f32, tag="et")
                nc.gpsimd.ap_gather(
                    out_ap=et[:],
                    in_ap=cur[:],
                    idxs_ap=ep[:],
                    channels=P,
                    num_elems=NG,
                    d=1,
                    num_idxs=rpck,
                )
                # merge buckets: Y.T[b, r] = sum_c et[16c+b, r]
                npsum = _ceil_to(rpck, 512) // 512
                ps = psp.tile([B, _ceil_to(rpck, 512)], f32, tag="ps")
                for m in range(npsum):
                    a, b_ = m * 512, min((m + 1) * 512, rpck)
                    nc.tensor.matmul(
                        out=ps[:, a:b_],
                        lhsT=sel_t[:],
                        rhs=et[:, a:b_, 0],
                        start=True,
                        stop=True,
                    )
                ysb = ysbp.tile([B, _ceil_to(rpc, 512)], f32, tag="ysb")
                nc.vector.tensor_copy(out=ysb[:, :rpck], in_=ps[:, :rpck])
                nc.sync.dma_start(y[:, rb : rb + rpck], ysb[:, :rpck])
                sbase += CL
    nc.compile()
    return nc


def _full_prep(X, vals, rows, cols, rows_per_core, rpc, n_cores):
    n_chunks = -(-rows_per_core // rpc)
    bounds = np.searchsorted(rows, np.arange(n_cores + 1) * rows_per_core)
    preps = []
    for n in range(n_cores):
        k0, k1 = bounds[n], bounds[n + 1]
        preps.append(
            _prep_core(
                (rows[k0:k1] - n * rows_per_core).astype(np.int64),
                cols[k0:k1].astype(np.int64),
                vals[k0:k1],
                rows_per_core,
                rpc,
            )
        )
    # shared per-chunk CLs = max over cores+buckets, mult of 64
    need = np.stack([p["need"] for p in preps])  # [cores, NBUK, n_chunks]
    cls_ = [
        int(_ceil_to(int(need[:, :, k].max()), 64)) for k in range(n_chunks)
    ]

    # X table: channel 16c+j (j<8) holds X[j, 8192c + e]
    T = np.zeros((P, BUK), np.float32)
    for c in range(NBUK):
        T[16 * c : 16 * c + 8, :] = X[:, BUK * c : BUK * (c + 1)]
    selm = np.zeros((P, B), np.float32)
    for c in range(NBUK):
        for j in range(B):
            selm[16 * c + j, j] = 1.0

    in_maps = []
    for n in range(n_cores):
        lay = _layout_core(preps[n], cls_, rows_per_core, rpc)
        in_maps.append(
            {
                "xt": T,
                "sel": selm,
                "idxw": lay["idxw"],
                "valsd": lay["valsd"],
                "rowidd": lay["rowid"],
                "epw": lay["epw"],
            }
        )
    return cls_, in_maps


def kernel(X, vals, rows, cols, _bcast_dma=True):
    X = np.asarray(X, np.float32)
    vals = np.asarray(vals, np.float32)
    rows = np.asarray(rows, np.int64)
    cols = np.asarray(cols, np.int64)
    rows_per_core = N_ROWS // N_CORES
    rpc = 768  # rows per chunk (mult of 16)

    cls_, in_maps = _full_prep(X, vals, rows, cols, rows_per_core, rpc, N_CORES)
    key = (tuple(cls_), rpc, rows_per_core, _bcast_dma)
    if key not in _compiled:
        _compiled[key] = _build_nc(cls_, rpc, rows_per_core, _bcast_dma)
    nc = _compiled[key]
    res = run_bass_kernel_spmd(nc, in_maps, core_ids=list(range(N_CORES)))
    Y = np.concatenate([res.results[n]["y"] for n in range(N_CORES)], axis=1)
    return Y.astype(np.float32)


# revision 7
# speedup vs baseline: 1.0874x; 1.0874x over previous
"""Batched sparse forward projection Y[b,r] = sum_k vals[k]*X[b,cols[k]] for rows[k]==r.

Strategy (8 NeuronCores, row-sharded):
- Each core owns a 16384-row slice; nnz slice via searchsorted (rows sorted).
- nnz bucketed by col>>13 into 8 buckets = 8 GPSIMD Q7 cores; stable bucketing
  keeps rows sorted per bucket.
- Gather via ap_gather: X table [128ch, 8192, 1] f32, channel 16c+j (j<8) holds
  X[j, 8192c+e]; per-Q7-core wrapped int16 index lists fetch all 8 batch values
  per nnz.
- Per chunk (768 output rows): contrib = gathered * vals (DVE), then a plain
  free-dim cumsum via tensor_tensor_scan, then a second ap_gather extracts the
  cumsum at each row's last-slot position (ends list, with a leading zero-slot);
  adjacent diffs give per-row/bucket/batch totals; a [128,8] selection matmul
  sums buckets into PSUM [8, rows].
- Empty rows need no slots: their end position inherits the previous row's,
  so the diff is zero.
"""

import numpy as np

import concourse.bass as bass
import concourse.mybir as mybir
import concourse.tile as tile
from concourse import bacc
from concourse.bass_utils import run_bass_kernel_spmd

B = 8
N_PIX = 65536
N_ROWS = 131072
N_CORES = 8
NBUK = 8
BUK = N_PIX // NBUK  # 8192
P = 128
RPC = 768  # rows per chunk

_compiled = {}


def _ceil_to(x, m):
    return -(-x // m) * m


def _prep_core(rows_l, cols_n, vals_n, rows_per_core, rpc):
    """Sort by (bucket, row); per-(bucket,chunk) slot needs (+1 zero slot)."""
    buk = (cols_n >> 13).astype(np.int64)
    e = (cols_n & (BUK - 1)).astype(np.int16)
    key = buk * rows_per_core + rows_l.astype(np.int64)
    perm = np.argsort(key, kind="stable")
    skey = key[perm]
    cnt = np.bincount(key, minlength=NBUK * rows_per_core).reshape(NBUK, rows_per_core)
    n_chunks = -(-rows_per_core // rpc)
    need = np.zeros((NBUK, n_chunks), np.int64)
    for k in range(n_chunks):
        r0, r1 = k * rpc, min((k + 1) * rpc, rows_per_core)
        need[:, k] = cnt[:, r0:r1].sum(axis=1) + 1  # +1 zero slot
    return {
        "perm": perm,
        "skey": skey,
        "cnt": cnt,
        "e": e,
        "vals": vals_n,
        "need": need,
        "n_chunks": n_chunks,
    }


def _layout_core(prep, cls_, rows_per_core, rpc):
    n_chunks = len(cls_)
    Ltot = int(sum(cls_))
    cnt = prep["cnt"]
    cbase = np.concatenate([[0], np.cumsum(cls_)]).astype(np.int64)

    skey, perm = prep["skey"], prep["perm"]
    c_sorted = skey // rows_per_core
    r_sorted = skey % rows_per_core
    chunk_id = r_sorted // rpc
    seg_key = c_sorted * n_chunks + chunk_id
    seg_cnt = np.bincount(seg_key, minlength=NBUK * n_chunks)
    seg_start = np.cumsum(seg_cnt) - seg_cnt
    rank = np.arange(skey.shape[0], dtype=np.int64) - seg_start[seg_key]
    pos = cbase[chunk_id] + 1 + rank  # +1 for the zero slot

    idx16 = np.full((NBUK, Ltot), -1, np.int16)
    valsd = np.zeros((NBUK, Ltot), np.float32)
    idx16[c_sorted, pos] = prep["e"][perm]
    valsd[c_sorted, pos] = prep["vals"][perm]

    # extraction lists per chunk: [0, ends(r0), ends(r0+1), ...] padded to rpc+16
    epl = rpc + 16
    epx = np.zeros((NBUK, n_chunks, epl), np.int16)
    ccnt = np.cumsum(cnt, axis=1)
    for k in range(n_chunks):
        r0, r1 = k * rpc, min((k + 1) * rpc, rows_per_core)
        prev = ccnt[:, r0 - 1] if r0 > 0 else np.zeros(NBUK, np.int64)
        ends = ccnt[:, r0:r1] - prev[:, None]  # last-slot pos (1-based w/ zero slot)
        epx[:, k, 1 : 1 + (r1 - r0)] = ends.astype(np.int16)
        epx[:, k, 1 + (r1 - r0) :] = ends[:, -1:].astype(np.int16)

    idxw = np.ascontiguousarray(
        idx16.reshape(NBUK, Ltot // 16, 16).transpose(0, 2, 1)
    ).reshape(NBUK * 16, Ltot // 16)
    epxw = np.ascontiguousarray(
        epx.reshape(NBUK, n_chunks * epl // 16, 16).transpose(0, 2, 1)
    ).reshape(NBUK * 16, n_chunks * epl // 16)
    return {"idxw": idxw, "valsd": valsd, "epxw": epxw}


def _build_nc(cls_, rpc, rows_per_core, repeat=1):
    n_chunks = len(cls_)
    Ltot = int(sum(cls_))
    epl = rpc + 16
    nc = bacc.Bacc("TRN2", target_bir_lowering=False, debug=False)
    f32, i16 = mybir.dt.float32, mybir.dt.int16

    xt = nc.dram_tensor("xt", [P, BUK], f32, kind="ExternalInput")
    sel = nc.dram_tensor("sel", [P, B], f32, kind="ExternalInput")
    idxw = nc.dram_tensor("idxw", [P, Ltot // 16], i16, kind="ExternalInput")
    valsd = nc.dram_tensor("valsd", [NBUK, Ltot], f32, kind="ExternalInput")
    epxw = nc.dram_tensor(
        "epxw", [P, n_chunks * epl // 16], i16, kind="ExternalInput"
    )
    y = nc.dram_tensor("y", [B, rows_per_core], f32, kind="ExternalOutput")

    CLmax = max(int(c) for c in cls_)

    with tile.TileContext(nc) as tc:
        with (
            tc.tile_pool(name="tabp", bufs=1) as tabp,
            tc.tile_pool(name="selp", bufs=1) as selp,
            tc.tile_pool(name="onep", bufs=1) as onep,
            tc.tile_pool(name="idxp", bufs=2) as idxp,
            tc.tile_pool(name="gtp", bufs=1) as gtp,
            tc.tile_pool(name="valp", bufs=1) as valp,
            tc.tile_pool(name="epp", bufs=2) as epp,
            tc.tile_pool(name="etp", bufs=2) as etp,
            tc.tile_pool(name="dtp", bufs=2) as dtp,
            tc.tile_pool(name="ysbp", bufs=2) as ysbp,
            tc.tile_pool(name="psp", bufs=2, space="PSUM") as psp,
        ):
            tab_t = tabp.tile([P, BUK, 1], f32)
            nc.sync.dma_start(tab_t[:, :, 0], xt[:])
            sel_t = selp.tile([P, B], f32)
            nc.sync.dma_start(sel_t[:], sel[:])
            ones_t = onep.tile([P, 1], f32)
            nc.vector.memset(ones_t[:], 1.0)

            for _rep in range(repeat):
                sbase = 0
                for k in range(n_chunks):
                    CL = int(cls_[k])
                    rb = k * rpc
                    rpck = min(rpc, rows_per_core - rb)

                    it = idxp.tile([P, CL // 16], i16, tag="idx")
                    nc.sync.dma_start(
                        it[:], idxw[:, sbase // 16 : (sbase + CL) // 16]
                    )
                    gt = gtp.tile([P, CL, 1], f32, tag="gt")
                    nc.gpsimd.ap_gather(
                        out_ap=gt[:],
                        in_ap=tab_t[:],
                        idxs_ap=it[:],
                        channels=P,
                        num_elems=BUK,
                        d=1,
                        num_idxs=CL,
                    )
                    vt = valp.tile([P, CL], f32, tag="val")
                    for c in range(NBUK):
                        src_v = bass.AP(valsd, c * Ltot + sbase, [[0, 16], [1, CL]])
                        nc.sync.dma_start(vt[16 * c : 16 * c + 16, :], src_v)
                    nc.vector.tensor_tensor(
                        out=gt[:, :, 0],
                        in0=gt[:, :, 0],
                        in1=vt[:],
                        op=mybir.AluOpType.mult,
                    )
                    # plain inclusive cumsum along the chunk (per partition)
                    nc.vector.tensor_tensor_scan(
                        out=gt[:, :, 0],
                        data0=ones_t[:].to_broadcast([P, CL]),
                        data1=gt[:, :, 0],
                        initial=0.0,
                        op0=mybir.AluOpType.mult,
                        op1=mybir.AluOpType.add,
                    )
                    # extract cumsum at [0, end(r0), end(r0+1), ...]
                    ep = epp.tile([P, epl // 16], i16, tag="ep")
                    nc.sync.dma_start(
                        ep[:], epxw[:, k * epl // 16 : (k + 1) * epl // 16]
                    )
                    et = etp.tile([P, epl, 1], f32, tag="et")
                    nc.gpsimd.ap_gather(
                        out_ap=et[:],
                        in_ap=gt[:],
                        idxs_ap=ep[:],
                        channels=P,
                        num_elems=CL,
                        d=1,
                        num_idxs=epl,
                    )
                    dt = dtp.tile([P, rpck], f32, tag="dt")
                    nc.vector.tensor_tensor(
                        out=dt[:],
                        in0=et[:, 1 : rpck + 1, 0],
                        in1=et[:, 0:rpck, 0],
                        op=mybir.AluOpType.subtract,
                    )
                    ps = psp.tile([B, _ceil_to(rpc, 512)], f32, tag="ps")
                    for m in range(_ceil_to(rpck, 512) // 512):
                        a, b_ = m * 512, min((m + 1) * 512, rpck)
                        nc.tensor.matmul(
                            out=ps[:, a:b_],
                            lhsT=sel_t[:],
                            rhs=dt[:, a:b_],
                            start=True,
                            stop=True,
                        )
                    ysb = ysbp.tile([B, _ceil_to(rpc, 512)], f32, tag="ysb")
                    nc.vector.tensor_copy(out=ysb[:, :rpck], in_=ps[:, :rpck])
                    nc.sync.dma_start(y[:, rb : rb + rpck], ysb[:, :rpck])
                    sbase += CL
    nc.compile()
    return nc


def _full_prep(X, vals, rows, cols, rows_per_core, rpc, n_cores):
    n_chunks = -(-rows_per_core // rpc)
    bounds = np.searchsorted(rows, np.arange(n_cores + 1) * rows_per_core)
    preps = []
    for n in range(n_cores):
        k0, k1 = bounds[n], bounds[n + 1]
        preps.append(
            _prep_core(
                (rows[k0:k1] - n * rows_per_core).astype(np.int64),
                cols[k0:k1].astype(np.int64),
                vals[k0:k1],
                rows_per_core,
                rpc,
            )
        )
    need = np.stack([p["need"] for p in preps])
    cls_ = [int(_ceil_to(int(need[:, :, k].max()), 64)) for k in range(n_chunks)]
    assert max(cls_) <= 16384, f"chunk too big: {max(cls_)}"

    T = np.zeros((P, BUK), np.float32)
    for c in range(NBUK):
        T[16 * c : 16 * c + 8, :] = X[:, BUK * c : BUK * (c + 1)]
    selm = np.zeros((P, B), np.float32)
    for c in range(NBUK):
        for j in range(B):
            selm[16 * c + j, j] = 1.0

    in_maps = []
    for n in range(n_cores):
        lay = _layout_core(preps[n], cls_, rows_per_core, rpc)
        in_maps.append(
            {
                "xt": T,
                "sel": selm,
                "idxw": lay["idxw"],
                "valsd": lay["valsd"],
                "epxw": lay["epxw"],
            }
        )
    return cls_, in_maps


def kernel(X, vals, rows, cols):
    X = np.asarray(X, np.float32)
    vals = np.asarray(vals, np.float32)
    rows = np.asarray(rows, np.int64)
    cols = np.asarray(cols, np.int64)
    rows_per_core = N_ROWS // N_CORES

    cls_, in_maps = _full_prep(X, vals, rows, cols, rows_per_core, RPC, N_CORES)
    key = (tuple(cls_), RPC, rows_per_core)
    if key not in _compiled:
        _compiled[key] = _build_nc(cls_, RPC, rows_per_core)
    nc = _compiled[key]
    res = run_bass_kernel_spmd(nc, in_maps, core_ids=list(range(N_CORES)))
    Y = np.concatenate([res.results[n]["y"] for n in range(N_CORES)], axis=1)
    return np.ascontiguousarray(Y, dtype=np.float32)


# revision 8
# speedup vs baseline: 43.0062x; 39.5510x over previous
"""Batched sparse forward projection Y[b,r] = sum_k vals[k]*X[b,cols[k]] for rows[k]==r.

Strategy (8 NeuronCores, row-sharded):
- Each core owns a 16384-row slice; nnz slice via searchsorted (rows sorted).
- nnz bucketed by col>>13 into 8 buckets = 8 GPSIMD Q7 cores; stable bucketing
  keeps rows sorted per bucket.
- Gather via ap_gather: X table [128ch, 8192, 1] f32, channel 16c+j (j<8) holds
  X[j, 8192c+e]; per-Q7-core wrapped int16 index lists fetch all 8 batch values
  per nnz.
- Per chunk (768 output rows): contrib = gathered * vals (DVE), then a plain
  free-dim cumsum via tensor_tensor_scan, then a second ap_gather extracts the
  cumsum at each row's last-slot position (ends list, with a leading zero-slot);
  adjacent diffs give per-row/bucket/batch totals; a [128,8] selection matmul
  sums buckets into PSUM [8, rows].
- Empty rows need no slots: their end position inherits the previous row's,
  so the diff is zero.
"""

import numpy as np

import concourse.bass as bass
import concourse.mybir as mybir
import concourse.tile as tile
from concourse import bacc
from concourse.bass_utils import run_bass_kernel_spmd

B = 8
N_PIX = 65536
N_ROWS = 131072
N_CORES = 8
NBUK = 8
BUK = N_PIX // NBUK  # 8192
P = 128
RPC = 768  # rows per chunk

_compiled = {}


def _ceil_to(x, m):
    return -(-x // m) * m


def _prep_core(rows_l, cols_n, vals_n, rows_per_core, rpc):
    """Sort by (bucket, row); per-(bucket,chunk) slot needs (+1 zero slot)."""
    buk = (cols_n >> 13).astype(np.int64)
    e = (cols_n & (BUK - 1)).astype(np.int16)
    key = buk * rows_per_core + rows_l.astype(np.int64)
    perm = np.argsort(key, kind="stable")
    skey = key[perm]
    cnt = np.bincount(key, minlength=NBUK * rows_per_core).reshape(NBUK, rows_per_core)
    n_chunks = -(-rows_per_core // rpc)
    need = np.zeros((NBUK, n_chunks), np.int64)
    for k in range(n_chunks):
        r0, r1 = k * rpc, min((k + 1) * rpc, rows_per_core)
        need[:, k] = cnt[:, r0:r1].sum(axis=1) + 1  # +1 zero slot
    return {
        "perm": perm,
        "skey": skey,
        "cnt": cnt,
        "e": e,
        "vals": vals_n,
        "need": need,
        "n_chunks": n_chunks,
    }


def _layout_core(prep, cls_, rows_per_core, rpc):
    n_chunks = len(cls_)
    Ltot = int(sum(cls_))
    cnt = prep["cnt"]
    cbase = np.concatenate([[0], np.cumsum(cls_)]).astype(np.int64)

    skey, perm = prep["skey"], prep["perm"]
    c_sorted = skey // rows_per_core
    r_sorted = skey % rows_per_core
    chunk_id = r_sorted // rpc
    seg_key = c_sorted * n_chunks + chunk_id
    seg_cnt = np.bincount(seg_key, minlength=NBUK * n_chunks)
    seg_start = np.cumsum(seg_cnt) - seg_cnt
    rank = np.arange(skey.shape[0], dtype=np.int64) - seg_start[seg_key]
    pos = cbase[chunk_id] + 1 + rank  # +1 for the zero slot

    idx16 = np.full((NBUK, Ltot), -1, np.int16)
    valsd = np.zeros((NBUK, Ltot), np.float32)
    idx16[c_sorted, pos] = prep["e"][perm]
    valsd[c_sorted, pos] = prep["vals"][perm]

    # extraction lists per chunk: [0, ends(r0), ends(r0+1), ...] padded to rpc+16
    epl = rpc + 16
    epx = np.zeros((NBUK, n_chunks, epl), np.int16)
    ccnt = np.cumsum(cnt, axis=1)
    for k in range(n_chunks):
        r0, r1 = k * rpc, min((k + 1) * rpc, rows_per_core)
        prev = ccnt[:, r0 - 1] if r0 > 0 else np.zeros(NBUK, np.int64)
        ends = ccnt[:, r0:r1] - prev[:, None]  # last-slot pos (1-based w/ zero slot)
        epx[:, k, 1 : 1 + (r1 - r0)] = ends.astype(np.int16)
        epx[:, k, 1 + (r1 - r0) :] = ends[:, -1:].astype(np.int16)

    idxw = np.ascontiguousarray(
        idx16.reshape(NBUK, Ltot // 16, 16).transpose(0, 2, 1)
    ).reshape(NBUK * 16, Ltot // 16)
    epxw = np.ascontiguousarray(
        epx.reshape(NBUK, n_chunks * epl // 16, 16).transpose(0, 2, 1)
    ).reshape(NBUK * 16, n_chunks * epl // 16)
    return {"idxw": idxw, "valsd": valsd, "epxw": epxw}


def _build_nc(cls_, rpc, rows_per_core, repeat=1):
    n_chunks = len(cls_)
    Ltot = int(sum(cls_))
    epl = rpc + 16
    nc = bacc.Bacc("TRN2", target_bir_lowering=False, debug=False)
    f32, i16 = mybir.dt.float32, mybir.dt.int16

    xt = nc.dram_tensor("xt", [P, BUK], f32, kind="ExternalInput")
    sel = nc.dram_tensor("sel", [P, B], f32, kind="ExternalInput")
    idxw = nc.dram_tensor("idxw", [P, Ltot // 16], i16, kind="ExternalInput")
    valsd = nc.dram_tensor("valsd", [NBUK, Ltot], f32, kind="ExternalInput")
    epxw = nc.dram_tensor(
        "epxw", [P, n_chunks * epl // 16], i16, kind="ExternalInput"
    )
    y = nc.dram_tensor("y", [B, rows_per_core], f32, kind="ExternalOutput")

    CLmax = max(int(c) for c in cls_)

    with tile.TileContext(nc) as tc:
        with (
            tc.tile_pool(name="tabp", bufs=1) as tabp,
            tc.tile_pool(name="selp", bufs=1) as selp,
            tc.tile_pool(name="onep", bufs=1) as onep,
            tc.tile_pool(name="idxp", bufs=2) as idxp,
            tc.tile_pool(name="gtp", bufs=1) as gtp,
            tc.tile_pool(name="valp", bufs=1) as valp,
            tc.tile_pool(name="epp", bufs=2) as epp,
            tc.tile_pool(name="etp", bufs=2) as etp,
            tc.tile_pool(name="dtp", bufs=2) as dtp,
            tc.tile_pool(name="ysbp", bufs=2) as ysbp,
            tc.tile_pool(name="psp", bufs=2, space="PSUM") as psp,
        ):
            tab_t = tabp.tile([P, BUK, 1], f32)
            nc.sync.dma_start(tab_t[:, :, 0], xt[:])
            sel_t = selp.tile([P, B], f32)
            nc.sync.dma_start(sel_t[:], sel[:])
            ones_t = onep.tile([P, 1], f32)
            nc.vector.memset(ones_t[:], 1.0)

            for _rep in range(repeat):
                sbase = 0
                for k in range(n_chunks):
                    CL = int(cls_[k])
                    rb = k * rpc
                    rpck = min(rpc, rows_per_core - rb)

                    it = idxp.tile([P, CL // 16], i16, tag="idx")
                    nc.sync.dma_start(
                        it[:], idxw[:, sbase // 16 : (sbase + CL) // 16]
                    )
                    gt = gtp.tile([P, CL, 1], f32, tag="gt")
                    nc.gpsimd.ap_gather(
                        out_ap=gt[:],
                        in_ap=tab_t[:],
                        idxs_ap=it[:],
                        channels=P,
                        num_elems=BUK,
                        d=1,
                        num_idxs=CL,
                    )
                    vt = valp.tile([P, CL], f32, tag="val")
                    for c in range(NBUK):
                        src_v = bass.AP(valsd, c * Ltot + sbase, [[0, 16], [1, CL]])
                        nc.sync.dma_start(vt[16 * c : 16 * c + 16, :], src_v)
                    nc.vector.tensor_tensor(
                        out=gt[:, :, 0],
                        in0=gt[:, :, 0],
                        in1=vt[:],
                        op=mybir.AluOpType.mult,
                    )
                    # plain inclusive cumsum along the chunk (per partition)
                    nc.vector.tensor_tensor_scan(
                        out=gt[:, :, 0],
                        data0=ones_t[:].to_broadcast([P, CL]),
                        data1=gt[:, :, 0],
                        initial=0.0,
                        op0=mybir.AluOpType.mult,
                        op1=mybir.AluOpType.add,
                    )
                    # extract cumsum at [0, end(r0), end(r0+1), ...]
                    ep = epp.tile([P, epl // 16], i16, tag="ep")
                    nc.sync.dma_start(
                        ep[:], epxw[:, k * epl // 16 : (k + 1) * epl // 16]
                    )
                    et = etp.tile([P, epl, 1], f32, tag="et")
                    nc.gpsimd.ap_gather(
                        out_ap=et[:],
                        in_ap=gt[:],
                        idxs_ap=ep[:],
                        channels=P,
                        num_elems=CL,
                        d=1,
                        num_idxs=epl,
                    )
                    dt = dtp.tile([P, rpck], f32, tag="dt")
                    nc.vector.tensor_tensor(
                        out=dt[:],
                        in0=et[:, 1 : rpck + 1, 0],
                        in1=et[:, 0:rpck, 0],
                        op=mybir.AluOpType.subtract,
                    )
                    ps = psp.tile([B, _ceil_to(rpc, 512)], f32, tag="ps")
                    for m in range(_ceil_to(rpck, 512) // 512):
                        a, b_ = m * 512, min((m + 1) * 512, rpck)
                        nc.tensor.matmul(
                            out=ps[:, a:b_],
                            lhsT=sel_t[:],
                            rhs=dt[:, a:b_],
                            start=True,
                            stop=True,
                        )
                    ysb = ysbp.tile([B, _ceil_to(rpc, 512)], f32, tag="ysb")
                    nc.vector.tensor_copy(out=ysb[:, :rpck], in_=ps[:, :rpck])
                    nc.sync.dma_start(y[:, rb : rb + rpck], ysb[:, :rpck])
                    sbase += CL
    nc.compile()
    return nc


def _full_prep(X, vals, rows, cols, rows_per_core, rpc, n_cores):
    n_chunks = -(-rows_per_core // rpc)
    bounds = np.searchsorted(rows, np.arange(n_cores + 1) * rows_per_core)
    preps = []
    for n in range(n_cores):
        k0, k1 = bounds[n], bounds[n + 1]
        preps.append(
            _prep_core(
                (rows[k0:k1] - n * rows_per_core).astype(np.int64),
                cols[k0:k1].astype(np.int64),
                vals[k0:k1],
                rows_per_core,
                rpc,
            )
        )
    need = np.stack([p["need"] for p in preps])
    cls_ = [int(_ceil_to(int(need[:, :, k].max()), 64)) for k in range(n_chunks)]
    assert max(cls_) <= 16384, f"chunk too big: {max(cls_)}"

    T = np.zeros((P, BUK), np.float32)
    for c in range(NBUK):
        T[16 * c : 16 * c + 8, :] = X[:, BUK * c : BUK * (c + 1)]
    selm = np.zeros((P, B), np.float32)
    for c in range(NBUK):
        for j in range(B):
            selm[16 * c + j, j] = 1.0

    in_maps = []
    for n in range(n_cores):
        lay = _layout_core(preps[n], cls_, rows_per_core, rpc)
        in_maps.append(
            {
                "xt": T,
                "sel": selm,
                "idxw": lay["idxw"],
                "valsd": lay["valsd"],
                "epxw": lay["epxw"],
            }
        )
    return cls_, in_maps


def kernel(X, vals, rows, cols):
    X = np.asarray(X, np.float32)
    vals = np.asarray(vals, np.float32)
    rows = np.asarray(rows, np.int64)
    cols = np.asarray(cols, np.int64)
    rows_per_core = N_ROWS // N_CORES

    rpc = RPC
    while True:
        try:
            cls_, in_maps = _full_prep(X, vals, rows, cols, rows_per_core, rpc, N_CORES)
            break
        except AssertionError:
            rpc //= 2  # denser-than-expected chunks: halve rows per chunk
            if rpc < 64:
                raise
    key = (tuple(cls_), rpc, rows_per_core)
    if key not in _compiled:
        _compiled[key] = _build_nc(cls_, rpc, rows_per_core)
    nc = _compiled[key]
    res = run_bass_kernel_spmd(nc, in_maps, core_ids=list(range(N_CORES)))
    Y = np.concatenate([res.results[n]["y"] for n in range(N_CORES)], axis=1)
    return np.ascontiguousarray(Y, dtype=np.float32)
